# revision 4
# baseline (speedup 1.0000x reference)
"""Trainium2 Bass kernel for nn_MultiHeadAttention (B=2, S=2048, D=1024, H=16).

Sharding: 8 cores = 2 batches x 4 head-groups (4 heads each, Megatron-style).
Each core computes q/k/v projections for its 4 heads (column-sharded),
attention, and a row-sharded partial of the output projection. The host
sums the 4 partials per batch and adds b_o.

Per-core layouts (all matmuls contract over the partition dim):
  xqT/xkT/xvT [D=1024, S=2048]  inputs, transposed on host
  qT, kT      [E=256, S=2048]   projections, e on partitions
  v           [S=2048, E=256]   natural, with a ones column per head (row sums)
  scoresT     [kpos, qpos]      per head; exp'd by ACT straight out of PSUM
  attn_outT   [E=256, S=2048]   unnormalized out^T + per-q reciprocal scaling
  partialT    [D=1024, S=2048]  output (host transposes + reduces)
"""

import numpy as np

import concourse.bacc as bacc
import concourse.mybir as mybir
from concourse.tile import TileContext
from concourse import bass_utils

F32 = mybir.dt.float32
F32R = mybir.dt.float32r
AF = mybir.ActivationFunctionType

P = 128
S = 2048
D = 1024
E = 256  # local output dims = 4 heads * 64
DK = 64
H_LOC = 4
N_CORES = 8
QB = 512  # q-block (PSUM bank width in fp32)
N_QB = S // QB  # 4
N_KC = S // P  # 16 kpos chunks
N_DO = D // P  # 8 contraction chunks for projections
N_EO = E // P  # 2
SCALE = 0.125  # 1/sqrt(DK)

_CACHE = {}


def _build_nc():
    nc = bacc.Bacc()

    xqT = nc.dram_tensor("xqT", [D, S], F32R, kind="ExternalInput")
    xkT = nc.dram_tensor("xkT", [D, S], F32R, kind="ExternalInput")
    xvT = nc.dram_tensor("xvT", [D, S], F32R, kind="ExternalInput")
    wqT = nc.dram_tensor("wqT", [D, E], F32R, kind="ExternalInput")
    wkT = nc.dram_tensor("wkT", [D, E], F32R, kind="ExternalInput")
    wvT = nc.dram_tensor("wvT", [D, E], F32R, kind="ExternalInput")
    woT = nc.dram_tensor("woT", [E, D], F32R, kind="ExternalInput")
    bq = nc.dram_tensor("bq", [E], F32, kind="ExternalInput")
    bk = nc.dram_tensor("bk", [E], F32, kind="ExternalInput")
    bv = nc.dram_tensor("bv", [E], F32, kind="ExternalInput")
    out = nc.dram_tensor("partialT", [D, S], F32, kind="ExternalOutput")

    xT = {"q": xqT, "k": xkT, "v": xvT}
    wT = {"q": wqT, "k": wkT, "v": wvT}
    bias = {"q": bq, "k": bk}

    with TileContext(nc) as tc:
        with (
            tc.tile_pool(name="const", bufs=1) as cpool,
            tc.tile_pool(name="big", bufs=1) as bigpool,
            tc.tile_pool(name="work", bufs=2) as work,
            tc.tile_pool(name="psum", bufs=2, space="PSUM") as psum,
            tc.tile_pool(name="dram", bufs=4, space="DRAM") as dram,
        ):
            # ---- constant loads ----
            w_sb = {}
            for t in ("q", "k", "v"):
                w_sb[t] = cpool.tile([P, N_DO, E], F32R, tag=f"w{t}", name=f"w{t}")
                nc.sync.dma_start(
                    out=w_sb[t], in_=wT[t].rearrange("(o p) e -> p o e", p=P)
                )
            wo_sb = cpool.tile([P, N_EO, D], F32R, tag="wo")
            nc.sync.dma_start(out=wo_sb, in_=woT.rearrange("(o p) e -> p o e", p=P))
            b_sb = {}
            for t in ("q", "k"):
                b_sb[t] = cpool.tile([P, N_EO], F32, tag=f"b{t}", name=f"b{t}")
                nc.sync.dma_start(
                    out=b_sb[t], in_=bias[t].rearrange("(o p) -> p o", p=P)
                )
            bv_bc = cpool.tile([P, E], F32, tag="bv")
            nc.sync.dma_start(out=bv_bc, in_=bv[None, :].to_broadcast([P, E]))

            # ---- persistent activations ----
            qT_sb = bigpool.tile([P, N_EO, S], F32R, tag="qT")
            kT_sb = bigpool.tile([P, N_EO, S], F32R, tag="kT")
            # v with a ones column per head: [s_part, kc, head, 65]
            v_sb = bigpool.tile([P, N_KC, H_LOC, DK + 1], F32R, tag="v")
            nc.vector.memset(v_sb[:, :, :, DK : DK + 1].bitcast(F32), 1.0)
            attn_sb = bigpool.tile([P, N_EO, S], F32R, tag="attn")

            def proj_qk(t):
                """qT/kT[e, s] = (wT.T @ xT) + bias, e on partitions."""
                dst = qT_sb if t == "q" else kT_sb
                for sb in range(N_QB):
                    xs = work.tile([P, N_DO, QB], F32R, tag="xs", bufs=3)
                    nc.sync.dma_start(
                        out=xs,
                        in_=xT[t].rearrange("(o p) s -> p o s", p=P)[
                            :, :, sb * QB : (sb + 1) * QB
                        ],
                    )
                    for eo in range(N_EO):
                        ps = psum.tile([P, QB], F32, tag="acc", bufs=4)
                        for do in range(N_DO):
                            nc.tensor.matmul(
                                ps[:, :],
                                w_sb[t][:, do, eo * P : (eo + 1) * P],
                                xs[:, do, :],
                                start=(do == 0),
                                stop=(do == N_DO - 1),
                            )
                        nc.vector.tensor_tensor(
                            dst[:, eo, sb * QB : (sb + 1) * QB],
                            ps[:, :],
                            b_sb[t][:, eo : eo + 1].to_broadcast([P, QB]),
                            mybir.AluOpType.add,
                        )

            def proj_v(sc):
                """v[s, e] rows sc*128..+128, written into the 65-strided layout."""
                xs = work.tile([P, N_DO, P], F32R, tag="xv", bufs=3)
                nc.sync.dma_start(
                    out=xs,
                    in_=xT["v"].rearrange("(o p) s -> p o s", p=P)[
                        :, :, sc * P : (sc + 1) * P
                    ],
                )
                ps = psum.tile([P, E], F32, tag="acc", bufs=4)
                for do in range(N_DO):
                    nc.tensor.matmul(
                        ps[:, :],
                        xs[:, do, :],
                        w_sb["v"][:, do, :],
                        start=(do == 0),
                        stop=(do == N_DO - 1),
                    )
                nc.vector.tensor_tensor(
                    v_sb[:, sc, :, 0:DK],
                    ps[:, :].rearrange("p (h d) -> p h d", h=H_LOC),
                    bv_bc.rearrange("p (h d) -> p h d", h=H_LOC),
                    mybir.AluOpType.add,
                )

            def head_slices(h):
                lo = DK * (h % 2)
                return slice(lo, lo + DK), h // 2

            def attention_qb(qb, pair, fold_v):
                hA, hB = 2 * pair, 2 * pair + 1
                accs = []
                for h in (hA, hB):
                    acc = psum.tile([P, QB], F32, tag="acc", bufs=4)
                    accs.append(acc)
                for kc in range(N_KC):
                    if fold_v:
                        proj_v(kc)
                    sc = psum.tile([P, 2 * QB], F32, tag="score", bufs=2)
                    for i, h in enumerate((hA, hB)):
                        prt, ec = head_slices(h)
                        nc.tensor.matmul(
                            sc[:, i * QB : (i + 1) * QB],
                            kT_sb[prt, ec, kc * P : (kc + 1) * P],
                            qT_sb[prt, ec, qb * QB : (qb + 1) * QB],
                            start=True,
                            stop=True,
                        )
                    pr = work.tile([P, 2 * QB], F32R, tag="probs", bufs=3)
                    nc.scalar.activation(pr[:, :], sc[:, :], AF.Exp, scale=SCALE)
                    for i, h in enumerate((hA, hB)):
                        nc.tensor.matmul(
                            accs[i][0 : DK + 1, :],
                            v_sb[:, kc, h, :],
                            pr[:, i * QB : (i + 1) * QB],
                            start=(kc == 0),
                            stop=(kc == N_KC - 1),
                        )
                # normalize: attn_outT[h] = acc[0:64] * (1 / acc[64]) bcast
                for i, h in enumerate((hA, hB)):
                    acc = accs[i]
                    rc = work.tile([1, QB], F32, tag="recip", bufs=4)
                    nc.vector.reciprocal(rc[0:1, :], acc[DK : DK + 1, :])
                    dsc = dram.tile([1, QB], F32)
                    nc.sync.dma_start(out=dsc[:, :], in_=rc[0:1, :])
                    rb = work.tile([DK, QB], F32, tag="rb", bufs=4)
                    nc.sync.dma_start(out=rb, in_=dsc.to_broadcast([DK, QB]))
                    prt, ec = head_slices(h)
                    nc.vector.tensor_tensor(
                        attn_sb[prt, ec, qb * QB : (qb + 1) * QB],
                        acc[0:DK, :],
                        rb[:, :],
                        mybir.AluOpType.mult,
                    )

            def out_proj_qb(qb):
                for eo in range(D // P):
                    ps = psum.tile([P, QB], F32, tag="acc", bufs=4)
                    for dlc in range(N_EO):
                        nc.tensor.matmul(
                            ps[:, :],
                            wo_sb[:, dlc, eo * P : (eo + 1) * P],
                            attn_sb[:, dlc, qb * QB : (qb + 1) * QB],
                            start=(dlc == 0),
                            stop=(dlc == N_EO - 1),
                        )
                    oe = work.tile([P, QB], F32, tag="oev", bufs=3)
                    nc.vector.tensor_copy(oe[:, :], ps[:, :])
                    nc.sync.dma_start(
                        out=out[eo * P : (eo + 1) * P, qb * QB : (qb + 1) * QB],
                        in_=oe[:, :],
                    )

            # ---- schedule ----
            proj_qk("k")
            proj_qk("q")
            for qb in range(N_QB):
                for pair in range(2):
                    attention_qb(qb, pair, fold_v=(qb == 0 and pair == 0))
                out_proj_qb(qb)

    nc.finalize()
    return nc


def _get_nc():
    if "nc" not in _CACHE:
        _CACHE["nc"] = _build_nc()
    return _CACHE["nc"]


def kernel(query, key, value, w_q, b_q, w_k, b_k, w_v, b_v, w_o, b_o):
    query = np.asarray(query, dtype=np.float32)
    key = np.asarray(key, dtype=np.float32)
    value = np.asarray(value, dtype=np.float32)
    w_q = np.asarray(w_q, dtype=np.float32)
    w_k = np.asarray(w_k, dtype=np.float32)
    w_v = np.asarray(w_v, dtype=np.float32)
    w_o = np.asarray(w_o, dtype=np.float32)
    b_q = np.asarray(b_q, dtype=np.float32)
    b_k = np.asarray(b_k, dtype=np.float32)
    b_v = np.asarray(b_v, dtype=np.float32)
    b_o = np.asarray(b_o, dtype=np.float32)

    nc = _get_nc()
    B = query.shape[0]
    n_hg = N_CORES // B  # 4 head groups

    in_maps = []
    for core in range(N_CORES):
        b, hg = divmod(core, n_hg)
        sl = slice(hg * E, (hg + 1) * E)
        in_maps.append(
            {
                "xqT": np.ascontiguousarray(query[b].T),
                "xkT": np.ascontiguousarray(key[b].T),
                "xvT": np.ascontiguousarray(value[b].T),
                "wqT": np.ascontiguousarray(w_q[sl, :].T),
                "wkT": np.ascontiguousarray(w_k[sl, :].T),
                "wvT": np.ascontiguousarray(w_v[sl, :].T),
                "woT": np.ascontiguousarray(w_o[:, sl].T),
                "bq": np.ascontiguousarray(b_q[sl]),
                "bk": np.ascontiguousarray(b_k[sl]),
                "bv": np.ascontiguousarray(b_v[sl]),
            }
        )

    res = bass_utils.run_bass_kernel_spmd(nc, in_maps, core_ids=list(range(N_CORES)))

    out = np.zeros((B, S, D), dtype=np.float32)
    for core in range(N_CORES):
        b = core // n_hg
        out[b] += res.results[core]["partialT"].T
    out += b_o
    return out


# revision 8
# speedup vs baseline: 1.1369x; 1.1369x over previous
"""Trainium2 Bass kernel for nn_MultiHeadAttention (B=2, S=2048, D=1024, H=16).

Sharding: 8 cores = 2 batches x 4 head-groups (4 heads each, Megatron-style).
Each core computes q/k/v projections for its 4 heads (column-sharded),
attention, and a row-sharded partial of the output projection. The host
sums the 4 partials per batch and adds b_o.

Per-core layouts (all matmuls contract over the partition dim):
  xqT/xkT/xvT [D=1024, S=2048]  inputs, transposed on host
  qT, kT      [E=256, S=2048]   projections, e on partitions
  v           [S=2048, E=256]   natural, with a ones column per head (row sums)
  scoresT     [kpos, qpos]      per head; exp'd by ACT straight out of PSUM
  attn_outT   [E=256, S=2048]   unnormalized out^T + per-q reciprocal scaling
  partialT    [D=1024, S=2048]  output (host transposes + reduces)
"""

import numpy as np

import concourse.bacc as bacc
import concourse.mybir as mybir
from concourse.tile import TileContext
from concourse import bass_utils

F32 = mybir.dt.float32
F32R = mybir.dt.float32r
AF = mybir.ActivationFunctionType

P = 128
S = 2048
D = 1024
E = 256  # local output dims = 4 heads * 64
DK = 64
H_LOC = 4
N_CORES = 8
QB = 512  # q-block (PSUM bank width in fp32)
N_QB = S // QB  # 4
N_KC = S // P  # 16 kpos chunks
N_DO = D // P  # 8 contraction chunks for projections
N_EO = E // P  # 2
SCALE = 0.125  # 1/sqrt(DK)

_CACHE = {}


def _build_nc():
    nc = bacc.Bacc()

    xqT = nc.dram_tensor("xqT", [D, S], F32R, kind="ExternalInput")
    xkT = nc.dram_tensor("xkT", [D, S], F32R, kind="ExternalInput")
    xvT = nc.dram_tensor("xvT", [D, S], F32R, kind="ExternalInput")
    wqT = nc.dram_tensor("wqT", [D, E], F32R, kind="ExternalInput")
    wkT = nc.dram_tensor("wkT", [D, E], F32R, kind="ExternalInput")
    wvT = nc.dram_tensor("wvT", [D, E], F32R, kind="ExternalInput")
    woT = nc.dram_tensor("woT", [E, D], F32R, kind="ExternalInput")
    bq = nc.dram_tensor("bq", [E], F32, kind="ExternalInput")
    bk = nc.dram_tensor("bk", [E], F32, kind="ExternalInput")
    bv = nc.dram_tensor("bv", [E], F32, kind="ExternalInput")
    out = nc.dram_tensor("partialT", [D, S], F32, kind="ExternalOutput")

    xT = {"q": xqT, "k": xkT, "v": xvT}
    wT = {"q": wqT, "k": wkT, "v": wvT}
    bias = {"q": bq, "k": bk}

    with TileContext(nc) as tc:
        with (
            tc.tile_pool(name="const", bufs=1) as cpool,
            tc.tile_pool(name="big", bufs=1) as bigpool,
            tc.tile_pool(name="work", bufs=2) as work,
            tc.tile_pool(name="psum", bufs=2, space="PSUM") as psum,
            tc.tile_pool(name="dram", bufs=4, space="DRAM") as dram,
        ):
            # ---- constants (loaded lazily, in consumption order) ----
            w_sb = {}
            for t in ("q", "k", "v"):
                w_sb[t] = cpool.tile([P, N_DO, E], F32R, tag=f"w{t}", name=f"w{t}")
            wo_sb = cpool.tile([P, N_EO, D], F32R, tag="wo")
            b_sb = {}
            for t in ("q", "k"):
                b_sb[t] = cpool.tile([P, N_EO], F32, tag=f"b{t}", name=f"b{t}")
            bv_bc = cpool.tile([P, E], F32, tag="bv")

            def load_consts(t):
                nc.sync.dma_start(
                    out=w_sb[t], in_=wT[t].rearrange("(o p) e -> p o e", p=P)
                )
                if t in ("q", "k"):
                    nc.sync.dma_start(
                        out=b_sb[t], in_=bias[t].rearrange("(o p) -> p o", p=P)
                    )
                else:
                    nc.sync.dma_start(out=bv_bc, in_=bv[None, :].to_broadcast([P, E]))

            # ---- persistent activations ----
            qT_sb = bigpool.tile([P, N_EO, S], F32R, tag="qT")
            kT_sb = bigpool.tile([P, N_EO, S], F32R, tag="kT")
            # v with a ones column per head: [s_part, kc, head, 65]
            v_sb = bigpool.tile([P, N_KC, H_LOC, DK + 1], F32R, tag="v")
            nc.vector.memset(v_sb[:, :, :, DK : DK + 1].bitcast(F32), 1.0)
            attn_sb = bigpool.tile([P, N_EO, S], F32R, tag="attn")

            def proj_qk(t, sb):
                """qT/kT[e, s-block sb] = (wT.T @ xT) + bias, e on partitions."""
                dst = qT_sb if t == "q" else kT_sb
                xs = work.tile([P, N_DO, QB], F32R, tag="xs", bufs=3)
                nc.sync.dma_start(
                    out=xs,
                    in_=xT[t].rearrange("(o p) s -> p o s", p=P)[
                        :, :, sb * QB : (sb + 1) * QB
                    ],
                )
                for eo in range(N_EO):
                    ps = psum.tile([P, QB], F32, tag="acc", bufs=4)
                    for do in range(N_DO):
                        nc.tensor.matmul(
                            ps[:, :],
                            w_sb[t][:, do, eo * P : (eo + 1) * P],
                            xs[:, do, :],
                            start=(do == 0),
                            stop=(do == N_DO - 1),
                        )
                    nc.vector.tensor_tensor(
                        dst[:, eo, sb * QB : (sb + 1) * QB],
                        ps[:, :],
                        b_sb[t][:, eo : eo + 1].to_broadcast([P, QB]),
                        mybir.AluOpType.add,
                    )

            def proj_v(sc):
                """v[s, e] rows sc*128..+128, written into the 65-strided layout."""
                xs = work.tile([P, N_DO, P], F32R, tag="xv", bufs=3)
                nc.sync.dma_start(
                    out=xs,
                    in_=xT["v"].rearrange("(o p) s -> p o s", p=P)[
                        :, :, sc * P : (sc + 1) * P
                    ],
                )
                ps = psum.tile([P, E], F32, tag="acc", bufs=4)
                for do in range(N_DO):
                    nc.tensor.matmul(
                        ps[:, :],
                        xs[:, do, :],
                        w_sb["v"][:, do, :],
                        start=(do == 0),
                        stop=(do == N_DO - 1),
                    )
                nc.vector.tensor_tensor(
                    v_sb[:, sc, :, 0:DK],
                    ps[:, :].rearrange("p (h d) -> p h d", h=H_LOC),
                    bv_bc.rearrange("p (h d) -> p h d", h=H_LOC),
                    mybir.AluOpType.add,
                )

            def head_slices(h):
                lo = DK * (h % 2)
                return slice(lo, lo + DK), h // 2

            def attention_qb(qb, pair, kc_hook=None):
                hA, hB = 2 * pair, 2 * pair + 1
                accs = []
                for h in (hA, hB):
                    acc = psum.tile([P, QB], F32, tag="acc", bufs=4)
                    accs.append(acc)
                for kc in range(N_KC):
                    if kc_hook is not None:
                        kc_hook(kc)
                    sc = psum.tile([P, 2 * QB], F32, tag="score", bufs=2)
                    for i, h in enumerate((hA, hB)):
                        prt, ec = head_slices(h)
                        nc.tensor.matmul(
                            sc[:, i * QB : (i + 1) * QB],
                            kT_sb[prt, ec, kc * P : (kc + 1) * P],
                            qT_sb[prt, ec, qb * QB : (qb + 1) * QB],
                            start=True,
                            stop=True,
                        )
                    pr = work.tile([P, 2 * QB], F32R, tag="probs", bufs=3)
                    nc.scalar.activation(pr[:, :], sc[:, :], AF.Exp, scale=SCALE)
                    for i, h in enumerate((hA, hB)):
                        nc.tensor.matmul(
                            accs[i][0 : DK + 1, :],
                            v_sb[:, kc, h, :],
                            pr[:, i * QB : (i + 1) * QB],
                            start=(kc == 0),
                            stop=(kc == N_KC - 1),
                        )
                # normalize: attn_outT[h] = acc[0:64] * (1 / acc[64]) bcast
                for i, h in enumerate((hA, hB)):
                    acc = accs[i]
                    rc = work.tile([1, QB], F32, tag="recip", bufs=4)
                    nc.vector.reciprocal(rc[0:1, :], acc[DK : DK + 1, :])
                    dsc = dram.tile([1, QB], F32)
                    nc.sync.dma_start(out=dsc[:, :], in_=rc[0:1, :])
                    rb = work.tile([DK, QB], F32, tag="rb", bufs=4)
                    nc.sync.dma_start(out=rb, in_=dsc.to_broadcast([DK, QB]))
                    prt, ec = head_slices(h)
                    nc.vector.tensor_tensor(
                        attn_sb[prt, ec, qb * QB : (qb + 1) * QB],
                        acc[0:DK, :],
                        rb[:, :],
                        mybir.AluOpType.mult,
                    )

            def out_proj_qb(qb):
                for eo in range(D // P):
                    ps = psum.tile([P, QB], F32, tag="acc", bufs=4)
                    for dlc in range(N_EO):
                        nc.tensor.matmul(
                            ps[:, :],
                            wo_sb[:, dlc, eo * P : (eo + 1) * P],
                            attn_sb[:, dlc, qb * QB : (qb + 1) * QB],
                            start=(dlc == 0),
                            stop=(dlc == N_EO - 1),
                        )
                    oe = work.tile([P, QB], F32, tag="oev", bufs=3)
                    nc.vector.tensor_copy(oe[:, :], ps[:, :])
                    nc.sync.dma_start(
                        out=out[eo * P : (eo + 1) * P, qb * QB : (qb + 1) * QB],
                        in_=oe[:, :],
                    )

            # ---- schedule ----
            # Preamble: just enough to start qb0/pair0 scores (kT sb0, qT sb0).
            load_consts("k")
            proj_qk("k", 0)
            load_consts("q")
            proj_qk("q", 0)
            load_consts("v")

            def qb0_pair0_hook(kc):
                # Fold remaining kT/qT s-blocks and all of the v projection
                # into the first attention loop so PE feeds ACT early.
                if 1 <= kc <= 3:
                    proj_qk("k", kc)
                    proj_qk("q", kc)
                proj_v(kc)

            wo_loaded = [False]

            for qb in range(N_QB):
                attention_qb(qb, 0, kc_hook=qb0_pair0_hook if qb == 0 else None)
                if qb == 0:
                    nc.sync.dma_start(
                        out=wo_sb, in_=woT.rearrange("(o p) e -> p o e", p=P)
                    )
                    wo_loaded[0] = True
                else:
                    out_proj_qb(qb - 1)  # deferred one qb: normalize chain hidden
                attention_qb(qb, 1)
            out_proj_qb(N_QB - 1)

    nc.finalize()
    return nc


def _get_nc():
    if "nc" not in _CACHE:
        _CACHE["nc"] = _build_nc()
    return _CACHE["nc"]


def kernel(query, key, value, w_q, b_q, w_k, b_k, w_v, b_v, w_o, b_o):
    query = np.asarray(query, dtype=np.float32)
    key = np.asarray(key, dtype=np.float32)
    value = np.asarray(value, dtype=np.float32)
    w_q = np.asarray(w_q, dtype=np.float32)
    w_k = np.asarray(w_k, dtype=np.float32)
    w_v = np.asarray(w_v, dtype=np.float32)
    w_o = np.asarray(w_o, dtype=np.float32)
    b_q = np.asarray(b_q, dtype=np.float32)
    b_k = np.asarray(b_k, dtype=np.float32)
    b_v = np.asarray(b_v, dtype=np.float32)
    b_o = np.asarray(b_o, dtype=np.float32)

    nc = _get_nc()
    B = query.shape[0]
    n_hg = N_CORES // B  # 4 head groups

    in_maps = []
    for core in range(N_CORES):
        b, hg = divmod(core, n_hg)
        sl = slice(hg * E, (hg + 1) * E)
        in_maps.append(
            {
                "xqT": np.ascontiguousarray(query[b].T),
                "xkT": np.ascontiguousarray(key[b].T),
                "xvT": np.ascontiguousarray(value[b].T),
                "wqT": np.ascontiguousarray(w_q[sl, :].T),
                "wkT": np.ascontiguousarray(w_k[sl, :].T),
                "wvT": np.ascontiguousarray(w_v[sl, :].T),
                "woT": np.ascontiguousarray(w_o[:, sl].T),
                "bq": np.ascontiguousarray(b_q[sl]),
                "bk": np.ascontiguousarray(b_k[sl]),
                "bv": np.ascontiguousarray(b_v[sl]),
            }
        )

    res = bass_utils.run_bass_kernel_spmd(nc, in_maps, core_ids=list(range(N_CORES)))

    out = np.zeros((B, S, D), dtype=np.float32)
    for core in range(N_CORES):
        b = core // n_hg
        out[b] += res.results[core]["partialT"].T
    out += b_o
    return out
